# revision 1
# baseline (speedup 1.0000x reference)
"""Trainium2 Bass kernel for LongcatFlash MoE experts (expert-parallel, 8 cores).

Problem: T=4096 tokens, H=1024, I=512, 32 routed + 8 zero (identity) experts,
top-4 routing, per-expert capacity 768.

Strategy (sharding_hint = expert parallelism):
  - Host: compute routing (stable sort by expert, capacity clip), permute
    tokens to their expert's core (the "all-to-all"), build per-core packed
    activation buffers with tokens on the GEMM free dimension.
  - Device (8 cores, SPMD): each core owns 4 routed experts; per expert run
    the gated MLP as tiled matmuls:
        gu[o, c]  = sum_h guT[h, o] * xT[h, c]      (o = 2I rows, c = tokens)
        mid[i, c] = silu(gate[i, c]) * up[i, c]
        y[h, c]   = sum_i dnT[i, h] * mid[i, c]
    Tokens live on the free dim (N <= 512 per matmul), weights are the
    stationary operand.
  - Host: gather per-assignment outputs, scale by router weight, scatter-add
    back per token, add the zero-expert weighted-identity term.
"""

import math
import os

import numpy as np

N_CORES = 8
R = 32  # routed experts
E_PER_CORE = R // N_CORES  # 4
CAPACITY = 768
H = 1024
I_DIM = 512
HT = H // 128  # 8 h-tiles
OT = 2 * I_DIM // 128  # 8 o-tiles of gate_up
IT = I_DIM // 128  # 4 i-tiles

# precision mode: "bf16" (fast, rel err ~4e-4) or "f32r" (fp32 storage,
# FP22 matmul, rel err ~3e-5 but ~1.4x slower: 2 PE cycles/row + 2x DMA)
PREC = os.environ.get("MOE_PREC", "bf16")

LAST_RUN = {}  # filled with exec_time_ns etc. for test harness use


def _route(idx, wts, n_tok):
    """Replicates the reference's capacity-buffer routing exactly.

    Returns per-assignment (expert, token, weight, slot, flat_index) for kept
    routed assignments, sorted by expert (stable), plus zero-expert weights.
    """
    K = idx.shape[1]
    A = n_tok * K
    flat_e = idx.reshape(-1).astype(np.int64)
    flat_t = np.repeat(np.arange(n_tok, dtype=np.int64), K)
    flat_w = wts.reshape(-1)
    order = np.argsort(flat_e, kind="stable")
    se = flat_e[order]
    st = flat_t[order]
    sw = flat_w[order]
    counts = np.bincount(flat_e, minlength=R + 8)
    starts = np.cumsum(counts) - counts
    pos = np.arange(A, dtype=np.int64) - starts[se]
    valid = (se < R) & (pos < CAPACITY)
    zero_w = np.where(idx >= R, wts, 0.0).sum(axis=1)
    return (
        se[valid],
        st[valid],
        sw[valid],
        pos[valid],
        order[valid],
        zero_w,
    )


def _chunks(S):
    n = (S + 511) // 512
    base = S // n
    rem = S - base * n
    out = []
    c0 = 0
    for i in range(n):
        cn = base + (1 if i < rem else 0)
        out.append((c0, cn))
        c0 += cn
    return out


_BUILD_CACHE = {}


def _build_bass(S, prec):
    import concourse.bacc as bacc
    import concourse.bass as bass
    import concourse.mybir as mybir
    from concourse import tile

    key = (S, prec)
    if key in _BUILD_CACHE:
        return _BUILD_CACHE[key]

    FT = mybir.dt.float32
    if prec == "bf16":
        dram_dt = mybir.dt.bfloat16
        sb_dt = mybir.dt.bfloat16
        mid_dt = mybir.dt.bfloat16
        out_dt = mybir.dt.bfloat16
    else:
        dram_dt = mybir.dt.float32r
        sb_dt = mybir.dt.float32r
        mid_dt = mybir.dt.float32r
        out_dt = mybir.dt.float32

    chunks = _chunks(S)

    nc = bacc.Bacc(None)
    xt_d = nc.declare_dram_parameter("xt", [E_PER_CORE, HT, 128, S], dram_dt, isOutput=False)
    gu_d = nc.declare_dram_parameter("guw", [E_PER_CORE, HT, 128, 1024], dram_dt, isOutput=False)
    dn_d = nc.declare_dram_parameter("dnw", [E_PER_CORE, IT, 128, 1024], dram_dt, isOutput=False)
    yt_d = nc.declare_dram_parameter("yt", [E_PER_CORE, 128, HT * S], out_dt, isOutput=True)

    silu_fn = mybir.ActivationFunctionType.Silu

    # bf16 tiles are half-size; the f32r fallback needs smaller pools to fit
    # SBUF (~192 KB/partition usable)
    gu_bufs = 2 * HT if prec == "bf16" else HT + IT
    y_bufs = 4 if prec == "bf16" else 2
    with tile.TileContext(nc) as tc:
        with (
            tc.tile_pool(name="xpool", bufs=2 * HT) as xpool,
            tc.tile_pool(name="gupool", bufs=gu_bufs) as gupool,
            tc.tile_pool(name="dnpool", bufs=2 * IT) as dnpool,
            tc.tile_pool(name="midpool", bufs=2 * IT * len(chunks)) as midpool,
            # sil tiles are ACT-written; unique slots (no reuse) keep the
            # Activation instruction at a single sync-wait (AC struct limit 1)
            tc.tile_pool(name="silpool", bufs=E_PER_CORE * IT * len(chunks)) as silpool,
            tc.tile_pool(name="ypool", bufs=y_bufs) as ypool,
            tc.tile_pool(name="pgpool", bufs=3, space="PSUM") as pgpool,
            tc.tile_pool(name="pupool", bufs=3, space="PSUM") as pupool,
            tc.tile_pool(name="pypool", bufs=2, space="PSUM") as pypool,
        ):
            for e in range(E_PER_CORE):
                # interleave x / gate_up stripe loads so the first matmul can
                # start as soon as stripe 0 lands; split issue across engines
                # (DMA trigger is ~0.6us each on one sequencer)
                xts = []
                guts = []
                for h in range(HT):
                    tx = xpool.tile([128, S], sb_dt, tag="xt")
                    nc.scalar.dma_start(tx[:], xt_d[e, h])
                    xts.append(tx)
                    tg = gupool.tile([128, 1024], sb_dt, tag="gu")
                    nc.sync.dma_start(tg[:], gu_d[e, h])
                    guts.append(tg)
                dnts = []
                for i in range(IT):
                    t = dnpool.tile([128, 1024], sb_dt, tag="dn")
                    nc.sync.dma_start(t[:], dn_d[e, i])
                    dnts.append(t)

                mids = {}
                ywide = ypool.tile([128, HT * S], out_dt, tag="yo")
                for ci, (c0, cn) in enumerate(chunks):
                    for oi in range(IT):
                        pg = pgpool.tile([128, cn], FT, tag="pg")
                        pu = pupool.tile([128, cn], FT, tag="pu")
                        for h in range(HT):
                            nc.tensor.matmul(
                                pg[:],
                                guts[h][:, oi * 128 : (oi + 1) * 128],
                                xts[h][:, c0 : c0 + cn],
                                start=(h == 0),
                                stop=(h == HT - 1),
                            )
                        for h in range(HT):
                            nc.tensor.matmul(
                                pu[:],
                                guts[h][:, (IT + oi) * 128 : (IT + oi + 1) * 128],
                                xts[h][:, c0 : c0 + cn],
                                start=(h == 0),
                                stop=(h == HT - 1),
                            )
                        sil = silpool.tile([128, cn], FT, tag="sil")
                        nc.scalar.activation(sil[:], pg[:], silu_fn)
                        m = midpool.tile([128, cn], mid_dt, tag="mid")
                        nc.vector.scalar_tensor_tensor(
                            m[:], pu[:], 1.0, sil[:],
                            mybir.AluOpType.mult, mybir.AluOpType.mult,
                        )
                        mids[(ci, oi)] = m
                    for h in range(HT):
                        py = pypool.tile([128, cn], FT, tag="py")
                        for i in range(IT):
                            nc.tensor.matmul(
                                py[:],
                                dnts[i][:, h * 128 : (h + 1) * 128],
                                mids[(ci, i)][:],
                                start=(i == 0),
                                stop=(i == IT - 1),
                            )
                        nc.vector.tensor_copy(
                            ywide[:, h * S + c0 : h * S + c0 + cn], py[:]
                        )
                        if ci == len(chunks) - 1 and h % 2 == 1:
                            h0 = h - 1
                            nc.gpsimd.dma_start(
                                yt_d[e, :, h0 * S : (h + 1) * S],
                                ywide[:, h0 * S : (h + 1) * S],
                            )

    nc.finalize()
    _BUILD_CACHE[key] = nc
    return nc


def _install_trace_shims():
    """Make trace=True usable in this image: provide the NTFF hook module and
    neutralize the artifact upload (no bucket access needed for local use)."""
    import sys
    import types

    try:
        import antenv.axon_hooks  # noqa: F401
    except ImportError:
        hook = None
        try:
            from trn_agent_boot.trn_boot import _ntff_profile_via_ctypes

            hook = _ntff_profile_via_ctypes("/opt/axon/libaxon_pjrt.so")
        except Exception:
            hook = None
        mod = types.ModuleType("antenv.axon_hooks")
        mod._hook = hook
        mod.get_axon_ntff_profile_hook = lambda: mod._hook
        mod.set_axon_ntff_profile_hook = lambda h: setattr(mod, "_hook", h)
        sys.modules["antenv.axon_hooks"] = mod

    import concourse.bass_utils as bu

    orig_upload = bu.upload_artifacts

    def safe_upload(tmpdir):
        try:
            return orig_upload(tmpdir)
        except Exception:
            return tmpdir

    bu.upload_artifacts = safe_upload


def kernel(**inputs):
    from concourse.bass_utils import run_bass_kernel_spmd

    hidden = np.ascontiguousarray(np.asarray(inputs["hidden_states"], dtype=np.float32))
    idx = np.asarray(inputs["top_k_index"]).astype(np.int64)
    wts = np.asarray(inputs["top_k_weights"], dtype=np.float32)
    gup = np.asarray(inputs["gate_up_proj"], dtype=np.float32)
    dnp = np.asarray(inputs["down_proj"], dtype=np.float32)

    n_tok = hidden.shape[0]
    K = idx.shape[1]

    ve, vt, vw, vp, va, zero_w = _route(idx, wts, n_tok)
    cnts = np.bincount(ve, minlength=R)
    maxc = int(cnts.max())
    # N multiple of 64 elements keeps the PE moving-operand stream at full
    # rate (440 measured 231 ns/MM vs 448 at 202 ns/MM)
    S = max(256, ((maxc + 63) // 64) * 64)

    if PREC == "bf16":
        import ml_dtypes

        io_np = ml_dtypes.bfloat16
    else:
        io_np = np.float32

    # per-expert slices in the expert-sorted assignment arrays
    estarts = np.cumsum(cnts) - cnts

    in_maps = []
    for c in range(N_CORES):
        xt = np.zeros((E_PER_CORE, HT, 128, S), dtype=io_np)
        for le in range(E_PER_CORE):
            ge = c * E_PER_CORE + le
            s0, cnt = estarts[ge], cnts[ge]
            if cnt == 0:
                continue
            toks = vt[s0 : s0 + cnt]
            # [cnt, H] -> [H, cnt] -> tiles [HT, 128, cnt]
            xbuf = hidden[toks].T.reshape(HT, 128, cnt)
            xt[le, :, :, :cnt] = xbuf.astype(io_np)
        guw = (
            gup[c * E_PER_CORE : (c + 1) * E_PER_CORE]
            .transpose(0, 2, 1)  # [4, H, 2I]
            .reshape(E_PER_CORE, HT, 128, 1024)
            .astype(io_np)
        )
        dnw = (
            dnp[c * E_PER_CORE : (c + 1) * E_PER_CORE]
            .transpose(0, 2, 1)  # [4, I, H]
            .reshape(E_PER_CORE, IT, 128, 1024)
            .astype(io_np)
        )
        in_maps.append({"xt": np.ascontiguousarray(xt),
                        "guw": np.ascontiguousarray(guw),
                        "dnw": np.ascontiguousarray(dnw)})

    nc = _build_bass(S, PREC)

    trace = bool(int(os.environ.get("KERNEL_TRACE", "0")))
    if trace:
        _install_trace_shims()
    res = run_bass_kernel_spmd(nc, in_maps, list(range(N_CORES)), trace=trace)
    LAST_RUN["exec_time_ns"] = res.exec_time_ns
    LAST_RUN["mean_exec_time_ns"] = res.mean_exec_time_ns
    LAST_RUN["instructions_and_trace"] = res.instructions_and_trace
    LAST_RUN["profile_json"] = res.profile_json

    # ---- combine on host ----
    out = hidden * zero_w[:, None].astype(np.float32)
    acc = np.zeros((n_tok * K, H), dtype=np.float32)
    for c in range(N_CORES):
        yt = np.asarray(res.results[c]["yt"]).astype(np.float32)  # [4, 128, HT*S]
        for le in range(E_PER_CORE):
            ge = c * E_PER_CORE + le
            s0, cnt = estarts[ge], cnts[ge]
            if cnt == 0:
                continue
            # [128, HT, S] -> [HT, 128, S] -> [H, S]
            y = yt[le].reshape(128, HT, S).transpose(1, 0, 2).reshape(H, S)[:, :cnt].T
            acc[va[s0 : s0 + cnt]] = y * vw[s0 : s0 + cnt, None]
    out += acc.reshape(n_tok, K, H).sum(axis=1)
    return out



# revision 3
# speedup vs baseline: 1.5894x; 1.5894x over previous
"""Trainium2 Bass kernel for LongcatFlash MoE experts (expert-parallel, 8 cores).

Problem: T=4096 tokens, H=1024, I=512, 32 routed + 8 zero (identity) experts,
top-4 routing, per-expert capacity 768.

Strategy (expert parallelism, fp8 DoubleRow matmuls):
  - Host: replicate the reference routing (stable sort by expert, capacity
    clip), permute tokens to their expert's core, quantize x / weights to
    fp8e4 (weights pre-scaled by SW=32 to clear the e4m3 subnormal range),
    build per-core packed buffers with tokens on the GEMM free dimension.
  - Slot layout: 4 expert slots per core with tiered widths shared across
    cores (slot j holds the experts ranked [8j, 8j+8) by load; width =
    ceil64 of the tier max). All cores run one SPMD program.
  - Device: per slot run the gated MLP as fp8 DoubleRow matmuls (each MM
    contracts 2 k-tiles = 256 rows):
        gu[o, c]  = sum_h guT[h, o] * xT[h, c]      (PSUM = SW * true)
        sil       = Silu(gu_gate / SW)              (scalar engine)
        mid       = (gu_up * SM/SW) * sil -> fp8    (vector engine, = SM*mid)
        y[h, c]   = sum_i dnT[i, h] * mid[i, c]     (PSUM = SW*SM * true)
    y is copied to SBUF as bf16 still scaled by SW*SM; the descale is folded
    into the host-side router-weight multiply (free).
  - Host: gather, scale by router weight / (SW*SM), scatter-add per token,
    add the zero-expert weighted-identity term.
"""

import os

import numpy as np

N_CORES = 8
R = 32  # routed experts
N_SLOTS = 4
CAPACITY = 768
H = 1024
I_DIM = 512
HT = H // 128  # 8 contraction tiles for gate_up
IT = I_DIM // 128  # 4 contraction tiles for down
WMAX = 512

SW = 32.0  # weight pre-scale (both projections)
SM = 8.0  # mid pre-scale for fp8 storage

LAST_RUN = {}  # filled with exec_time_ns etc. for test harness use


def _route(idx, wts, n_tok):
    """Replicates the reference's capacity-buffer routing exactly.

    Returns per-assignment (expert, token, weight, slot, flat_index) for kept
    routed assignments, sorted by expert (stable), plus zero-expert weights.
    """
    K = idx.shape[1]
    A = n_tok * K
    flat_e = idx.reshape(-1).astype(np.int64)
    flat_t = np.repeat(np.arange(n_tok, dtype=np.int64), K)
    flat_w = wts.reshape(-1)
    order = np.argsort(flat_e, kind="stable")
    se = flat_e[order]
    st = flat_t[order]
    sw = flat_w[order]
    counts = np.bincount(flat_e, minlength=R + 8)
    starts = np.cumsum(counts) - counts
    pos = np.arange(A, dtype=np.int64) - starts[se]
    valid = (se < R) & (pos < CAPACITY)
    zero_w = np.where(idx >= R, wts, 0.0).sum(axis=1)
    return (
        se[valid],
        st[valid],
        sw[valid],
        pos[valid],
        order[valid],
        zero_w,
    )


_BUILD_CACHE = {}


def _build_bass(widths):
    import concourse.bacc as bacc
    import concourse.bass as bass
    import concourse.mybir as mybir
    from concourse import tile

    key = tuple(widths)
    if key in _BUILD_CACHE:
        return _BUILD_CACHE[key]

    FT = mybir.dt.float32
    F8 = mybir.dt.float8e4
    BF = mybir.dt.bfloat16
    DR = mybir.MatmulPerfMode.DoubleRow
    silu_fn = mybir.ActivationFunctionType.Silu

    WTOT = sum(widths)
    xoffs = [sum(widths[:s]) for s in range(N_SLOTS)]

    nc = bacc.Bacc(None)
    xt_d = nc.declare_dram_parameter("xt", [HT, 128, WTOT], F8, isOutput=False)
    gu_d = nc.declare_dram_parameter("guw", [N_SLOTS, 128, HT, 1024], F8, isOutput=False)
    dn_d = nc.declare_dram_parameter("dnw", [N_SLOTS, 128, IT, 1024], F8, isOutput=False)
    yt_d = nc.declare_dram_parameter("yt", [N_SLOTS, 128, HT, WMAX], BF, isOutput=True)

    # A-phase oi order: last-emitted STT feeds the D-phase group emitted last
    A_ORDER = (2, 3, 0, 1)

    with tile.TileContext(nc) as tc:
        with (
            tc.tile_pool(name="xpool", bufs=1) as xpool,
            tc.tile_pool(name="gupool", bufs=N_SLOTS) as gupool,
            tc.tile_pool(name="dnpool", bufs=N_SLOTS) as dnpool,
            tc.tile_pool(name="midpool", bufs=2) as midpool,
            tc.tile_pool(name="silpool", bufs=8) as silpool,
            tc.tile_pool(name="ypool", bufs=2) as ypool,
            tc.tile_pool(name="pgpool", bufs=2, space="PSUM") as pgpool,
            tc.tile_pool(name="pupool", bufs=2, space="PSUM") as pupool,
            tc.tile_pool(name="pypool", bufs=2, space="PSUM") as pypool,
        ):
            # ---- DMA in: everything up front, slot 0 first ----
            xt = xpool.tile([128, HT, WTOT], F8, tag="xt")
            guts = [gupool.tile([128, HT, 1024], F8, tag="gu", name=f"gut{s}") for s in range(N_SLOTS)]
            dnts = [dnpool.tile([128, IT, 1024], F8, tag="dn", name=f"dnt{s}") for s in range(N_SLOTS)]
            # slot0's gu in h-pair quarters interleaved with x so the first
            # matmuls start as soon as quarter 0 lands
            for q in range(4):
                nc.scalar.dma_start(xt[:, 2 * q, :], xt_d[2 * q])
                nc.scalar.dma_start(xt[:, 2 * q + 1, :], xt_d[2 * q + 1])
                nc.sync.dma_start(guts[0][:, 2 * q : 2 * q + 2, :], gu_d[0][:, 2 * q : 2 * q + 2, :])
            nc.sync.dma_start(dnts[0][:], dn_d[0])
            for s in range(1, N_SLOTS):
                nc.sync.dma_start(guts[s][:], gu_d[s])
                nc.sync.dma_start(dnts[s][:], dn_d[s])

            # ---- compute ----
            for s in range(N_SLOTS):
                W = widths[s]
                x0 = xoffs[s]
                gut, dnt = guts[s], dnts[s]

                # A phase: gate/up projections -> sil (scalar) -> mid (vector)
                mid = midpool.tile([128, IT, WMAX], F8, tag="mid")
                for oi in A_ORDER:
                    pg = pgpool.tile([128, WMAX], FT, tag="pg")
                    pu = pupool.tile([128, WMAX], FT, tag="pu")
                    for j in range(4):
                        nc.tensor.matmul(
                            pg[:, :W],
                            gut[:, 2 * j : 2 * j + 2, oi * 128 : (oi + 1) * 128],
                            xt[:, 2 * j : 2 * j + 2, x0 : x0 + W],
                            start=(j == 0),
                            stop=(j == 3),
                            perf_mode=DR,
                        )
                    for j in range(4):
                        nc.tensor.matmul(
                            pu[:, :W],
                            gut[:, 2 * j : 2 * j + 2, 512 + oi * 128 : 512 + (oi + 1) * 128],
                            xt[:, 2 * j : 2 * j + 2, x0 : x0 + W],
                            start=(j == 0),
                            stop=(j == 3),
                            perf_mode=DR,
                        )
                    sil = silpool.tile([128, WMAX], FT, tag="sil")
                    nc.scalar.activation(sil[:, :W], pg[:, :W], silu_fn, scale=1.0 / SW)
                    nc.vector.scalar_tensor_tensor(
                        mid[:, oi, :W], pu[:, :W], SM / SW, sil[:, :W],
                        mybir.AluOpType.mult, mybir.AluOpType.mult,
                    )

                # D phase: down projection; h-pairs share a 2-bank PSUM tile
                ywide = ypool.tile([128, HT, WMAX], BF, tag="yo")
                for hp in range(4):
                    py = pypool.tile([128, 2, WMAX], FT, tag="py")
                    for h01 in range(2):
                        h = 2 * hp + h01
                        # emit the j=1 group first: its mids (oi 2,3) are
                        # produced first by A_ORDER
                        for j in (1, 0):
                            nc.tensor.matmul(
                                py[:, h01, :W],
                                dnt[:, 2 * j : 2 * j + 2, h * 128 : (h + 1) * 128],
                                mid[:, 2 * j : 2 * j + 2, :W],
                                start=(j == 1),
                                stop=(j == 0),
                                perf_mode=DR,
                            )
                    dst = ywide[:, 2 * hp : 2 * hp + 2, :W]
                    if hp % 2 == 0:
                        nc.vector.tensor_copy(dst, py[:, :, :W])
                    else:
                        nc.scalar.copy(dst, py[:, :, :W])
                    if hp == 1:
                        nc.gpsimd.dma_start(yt_d[s][:, 0:4, :], ywide[:, 0:4, :])
                    elif hp == 3:
                        nc.gpsimd.dma_start(yt_d[s][:, 4:8, :], ywide[:, 4:8, :])

    nc.finalize()
    _BUILD_CACHE[key] = nc
    return nc


def _install_trace_shims():
    """Make trace=True usable in this image: provide the NTFF hook module and
    neutralize the artifact upload (no bucket access needed for local use)."""
    import sys
    import types

    try:
        import antenv.axon_hooks  # noqa: F401
    except ImportError:
        hook = None
        try:
            from trn_agent_boot.trn_boot import _ntff_profile_via_ctypes

            hook = _ntff_profile_via_ctypes("/opt/axon/libaxon_pjrt.so")
        except Exception:
            hook = None
        mod = types.ModuleType("antenv.axon_hooks")
        mod._hook = hook
        mod.get_axon_ntff_profile_hook = lambda: mod._hook
        mod.set_axon_ntff_profile_hook = lambda h: setattr(mod, "_hook", h)
        sys.modules["antenv.axon_hooks"] = mod

    import concourse.bass_utils as bu

    orig_upload = bu.upload_artifacts

    def safe_upload(tmpdir):
        try:
            return orig_upload(tmpdir)
        except Exception:
            return tmpdir
    bu.upload_artifacts = safe_upload


def kernel(**inputs):
    import ml_dtypes

    from concourse.bass_utils import run_bass_kernel_spmd

    F8NP = ml_dtypes.float8_e4m3

    hidden = np.ascontiguousarray(np.asarray(inputs["hidden_states"], dtype=np.float32))
    idx = np.asarray(inputs["top_k_index"]).astype(np.int64)
    wts = np.asarray(inputs["top_k_weights"], dtype=np.float32)
    gup = np.asarray(inputs["gate_up_proj"], dtype=np.float32)
    dnp = np.asarray(inputs["down_proj"], dtype=np.float32)

    n_tok = hidden.shape[0]
    K = idx.shape[1]

    ve, vt, vw, vp, va, zero_w = _route(idx, wts, n_tok)
    cnts = np.bincount(ve, minlength=R)
    estarts = np.cumsum(cnts) - cnts

    # tiered slot widths: slot j holds experts ranked [8j, 8j+8) by load
    rank = np.argsort(-cnts, kind="stable")
    widths = []
    for j in range(N_SLOTS):
        w = int(((cnts[rank[8 * j]] + 63) // 64) * 64)
        widths.append(max(64, min(WMAX, w)))
    assert cnts.max() <= WMAX, "expert load exceeds 512; unsupported"
    WTOT = sum(widths)
    xoffs = [sum(widths[:s]) for s in range(N_SLOTS)]
    # slot_expert[c][s] = global expert id
    slot_expert = [[int(rank[8 * s + c]) for s in range(N_SLOTS)] for c in range(N_CORES)]

    xq = hidden.astype(F8NP)  # [T, H] quantized once

    in_maps = []
    for c in range(N_CORES):
        xt = np.zeros((HT, 128, WTOT), dtype=F8NP)
        guw = np.empty((N_SLOTS, 128, HT, 1024), dtype=F8NP)
        dnw = np.empty((N_SLOTS, 128, IT, 1024), dtype=F8NP)
        for s in range(N_SLOTS):
            ge = slot_expert[c][s]
            s0, cnt = estarts[ge], cnts[ge]
            if cnt:
                toks = vt[s0 : s0 + cnt]
                # [cnt, H] -> [H, cnt] -> [HT, 128, cnt]
                xb = xq[toks].T.reshape(HT, 128, cnt)
                xt[:, :, xoffs[s] : xoffs[s] + cnt] = xb
            # guT[h, m] tiles: [128p, HT, 1024m]
            guw[s] = (gup[ge].T.reshape(HT, 128, 1024) * SW).astype(F8NP).transpose(1, 0, 2)
            # dnT[i, h] tiles: [128p, IT, 1024h]
            dnw[s] = (dnp[ge].T.reshape(IT, 128, 1024) * SW).astype(F8NP).transpose(1, 0, 2)
        in_maps.append({
            "xt": np.ascontiguousarray(xt),
            "guw": np.ascontiguousarray(guw),
            "dnw": np.ascontiguousarray(dnw),
        })

    nc = _build_bass(widths)

    trace = bool(int(os.environ.get("KERNEL_TRACE", "0")))
    if trace:
        _install_trace_shims()
    res = run_bass_kernel_spmd(nc, in_maps, list(range(N_CORES)), trace=trace)
    LAST_RUN["exec_time_ns"] = res.exec_time_ns
    LAST_RUN["mean_exec_time_ns"] = res.mean_exec_time_ns
    LAST_RUN["instructions_and_trace"] = res.instructions_and_trace
    LAST_RUN["profile_json"] = res.profile_json

    # ---- combine on host (descale by SW*SM folded into router weight) ----
    out = hidden * zero_w[:, None].astype(np.float32)
    acc = np.zeros((n_tok * K, H), dtype=np.float32)
    descale = 1.0 / (SW * SM)
    for c in range(N_CORES):
        yt = np.asarray(res.results[c]["yt"]).astype(np.float32)  # [4, 128, HT, 512]
        for s in range(N_SLOTS):
            ge = slot_expert[c][s]
            s0, cnt = estarts[ge], cnts[ge]
            if cnt == 0:
                continue
            y = yt[s].transpose(1, 0, 2).reshape(H, WMAX)[:, :cnt].T  # [cnt, H]
            acc[va[s0 : s0 + cnt]] = y * (vw[s0 : s0 + cnt, None] * descale)
    out += acc.reshape(n_tok, K, H).sum(axis=1)
    return out


# revision 5
# speedup vs baseline: 1.7529x; 1.1029x over previous
"""Trainium2 Bass kernel for LongcatFlash MoE experts (expert-parallel, 8 cores).

Problem: T=4096 tokens, H=1024, I=512, 32 routed + 8 zero (identity) experts,
top-4 routing, per-expert capacity 768.

Strategy (expert parallelism, fp8 DoubleRow matmuls):
  - Host: replicate the reference routing (stable sort by expert, capacity
    clip), permute tokens to their expert's core, quantize x / weights to
    fp8e4 (weights pre-scaled by SW=32 to clear the e4m3 subnormal range),
    build per-core packed buffers with tokens on the GEMM free dimension.
  - Slot layout: 4 expert slots per core with tiered widths shared across
    cores (slot j holds the experts ranked [8j, 8j+8) by load; width =
    ceil16 of the tier max — moving-operand segments need 16B alignment
    only). All cores run one SPMD program.
  - Device: per slot run the gated MLP as fp8 DoubleRow matmuls (each MM
    contracts 2 k-tiles = 256 rows):
        gu[o, c]  = sum_h guT[h, o] * xT[h, c]      (PSUM = SW * true)
        sil       = Silu(gu_gate / SW)              (scalar engine)
        mid       = (gu_up * SM/SW) * sil -> fp8    (vector engine, = SM*mid)
        y[h, c]   = sum_i dnT[i, h] * mid[i, c]     (PSUM = SW*SM * true)
    y is copied to SBUF as bf16 still scaled by SW*SM; the descale is folded
    into the host-side router-weight multiply (free).
  - x and slot0's gate_up live in per-h-pair tiles so the first matmul only
    waits on its own pair's DMA, not the whole activation load.
  - Host: gather, scale by router weight / (SW*SM), scatter-add per token,
    add the zero-expert weighted-identity term.
"""

import os

import numpy as np

N_CORES = 8
R = 32  # routed experts
N_SLOTS = 4
CAPACITY = 768
H = 1024
I_DIM = 512
HT = H // 128  # 8 contraction tiles for gate_up
IT = I_DIM // 128  # 4 contraction tiles for down
WMAX = 512

SW = 32.0  # weight pre-scale (both projections)
SM = 8.0  # mid pre-scale for fp8 storage

LAST_RUN = {}  # filled with exec_time_ns etc. for test harness use


def _route(idx, wts, n_tok):
    """Replicates the reference's capacity-buffer routing exactly.

    Returns per-assignment (expert, token, weight, slot, flat_index) for kept
    routed assignments, sorted by expert (stable), plus zero-expert weights.
    """
    K = idx.shape[1]
    A = n_tok * K
    flat_e = idx.reshape(-1).astype(np.int64)
    flat_t = np.repeat(np.arange(n_tok, dtype=np.int64), K)
    flat_w = wts.reshape(-1)
    order = np.argsort(flat_e, kind="stable")
    se = flat_e[order]
    st = flat_t[order]
    sw = flat_w[order]
    counts = np.bincount(flat_e, minlength=R + 8)
    starts = np.cumsum(counts) - counts
    pos = np.arange(A, dtype=np.int64) - starts[se]
    valid = (se < R) & (pos < CAPACITY)
    zero_w = np.where(idx >= R, wts, 0.0).sum(axis=1)
    return (
        se[valid],
        st[valid],
        sw[valid],
        pos[valid],
        order[valid],
        zero_w,
    )


_BUILD_CACHE = {}


def _build_bass(widths):
    import concourse.bacc as bacc
    import concourse.bass as bass
    import concourse.mybir as mybir
    from concourse import tile

    key = tuple(widths)
    if key in _BUILD_CACHE:
        return _BUILD_CACHE[key]

    FT = mybir.dt.float32
    F8 = mybir.dt.float8e4
    BF = mybir.dt.bfloat16
    DR = mybir.MatmulPerfMode.DoubleRow
    silu_fn = mybir.ActivationFunctionType.Silu

    WTOT = sum(widths)
    xoffs = [sum(widths[:s]) for s in range(N_SLOTS)]

    nc = bacc.Bacc(None)
    # x pair-major: [pair, 128, h01, WTOT]
    xt_d = nc.declare_dram_parameter("xt", [4, 128, 2, WTOT], F8, isOutput=False)
    gu_d = nc.declare_dram_parameter("guw", [N_SLOTS, 128, HT, 1024], F8, isOutput=False)
    dn_d = nc.declare_dram_parameter("dnw", [N_SLOTS, 128, IT, 1024], F8, isOutput=False)
    yt_d = nc.declare_dram_parameter("yt", [N_SLOTS, 128, HT, WMAX], BF, isOutput=True)

    # A-phase oi order: last-emitted STT feeds the D-phase group emitted last
    A_ORDER = (2, 3, 0, 1)

    with tile.TileContext(nc) as tc:
        with (
            tc.tile_pool(name="xpool", bufs=4) as xpool,
            tc.tile_pool(name="gu0pool", bufs=4) as gu0pool,
            tc.tile_pool(name="gupool", bufs=N_SLOTS - 1) as gupool,
            tc.tile_pool(name="dnpool", bufs=N_SLOTS) as dnpool,
            tc.tile_pool(name="midpool", bufs=2) as midpool,
            tc.tile_pool(name="silpool", bufs=8) as silpool,
            tc.tile_pool(name="ypool", bufs=2) as ypool,
            tc.tile_pool(name="pgpool", bufs=2, space="PSUM") as pgpool,
            tc.tile_pool(name="pupool", bufs=2, space="PSUM") as pupool,
            tc.tile_pool(name="pypool", bufs=3, space="PSUM") as pypool,
        ):
            # ---- DMA in: everything up front, slot-0 pair 0 first ----
            xps = [xpool.tile([128, 2, WTOT], F8, tag="xp", name=f"xp{q}") for q in range(4)]
            gu0s = [gu0pool.tile([128, 2, 1024], F8, tag="gu0", name=f"gu0p{q}") for q in range(4)]
            dnts = [dnpool.tile([128, IT, 1024], F8, tag="dn", name=f"dnt{s}") for s in range(N_SLOTS)]
            guts = [None] + [
                gupool.tile([128, HT, 1024], F8, tag="gu", name=f"gut{s}") for s in range(1, N_SLOTS)
            ]
            for q in range(4):
                eng = nc.scalar if q % 2 == 0 else nc.gpsimd
                eng.dma_start(xps[q][:], xt_d[q])
                nc.sync.dma_start(gu0s[q][:], gu_d[0][:, 2 * q : 2 * q + 2, :])
            nc.sync.dma_start(dnts[0][:], dn_d[0])
            for s in range(1, N_SLOTS):
                nc.sync.dma_start(guts[s][:], gu_d[s])
                nc.sync.dma_start(dnts[s][:], dn_d[s])

            # ---- compute ----
            for s in range(N_SLOTS):
                W = widths[s]
                x0 = xoffs[s]
                dnt = dnts[s]

                def gu_slice(j, c0, c1, _s=s):
                    if _s == 0:
                        return gu0s[j][:, :, c0:c1]
                    return guts[_s][:, 2 * j : 2 * j + 2, c0:c1]

                # A phase: gate/up projections -> sil (scalar) -> mid (vector)
                mid = midpool.tile([128, IT, WMAX], F8, tag="mid")
                for oi in A_ORDER:
                    pg = pgpool.tile([128, WMAX], FT, tag="pg")
                    pu = pupool.tile([128, WMAX], FT, tag="pu")
                    for j in range(4):
                        nc.tensor.matmul(
                            pg[:, :W],
                            gu_slice(j, oi * 128, (oi + 1) * 128),
                            xps[j][:, :, x0 : x0 + W],
                            start=(j == 0),
                            stop=(j == 3),
                            perf_mode=DR,
                        )
                    for j in range(4):
                        nc.tensor.matmul(
                            pu[:, :W],
                            gu_slice(j, 512 + oi * 128, 512 + (oi + 1) * 128),
                            xps[j][:, :, x0 : x0 + W],
                            start=(j == 0),
                            stop=(j == 3),
                            perf_mode=DR,
                        )
                    sil = silpool.tile([128, WMAX], FT, tag="sil")
                    nc.scalar.activation(sil[:, :W], pg[:, :W], silu_fn, scale=1.0 / SW)
                    nc.vector.scalar_tensor_tensor(
                        mid[:, oi, :W], pu[:, :W], SM / SW, sil[:, :W],
                        mybir.AluOpType.mult, mybir.AluOpType.mult,
                    )

                # D phase: down projection, per-h PSUM groups and copies
                ywide = ypool.tile([128, HT, WMAX], BF, tag="yo")
                for h in range(HT):
                    py = pypool.tile([128, WMAX], FT, tag="py")
                    # j=1 first: its mids (oi 2,3) are produced first by A_ORDER
                    for j in (1, 0):
                        nc.tensor.matmul(
                            py[:, :W],
                            dnt[:, 2 * j : 2 * j + 2, h * 128 : (h + 1) * 128],
                            mid[:, 2 * j : 2 * j + 2, :W],
                            start=(j == 1),
                            stop=(j == 0),
                            perf_mode=DR,
                        )
                    dst = ywide[:, h, :W]
                    if h % 2 == 0:
                        nc.vector.tensor_copy(dst, py[:, :W])
                    else:
                        nc.scalar.copy(dst, py[:, :W])
                    if s < N_SLOTS - 1:
                        if h == 3:
                            nc.gpsimd.dma_start(yt_d[s][:, 0:4, :], ywide[:, 0:4, :])
                        elif h == 7:
                            nc.gpsimd.dma_start(yt_d[s][:, 4:8, :], ywide[:, 4:8, :])
                    elif h % 2 == 1:
                        # last slot: quarter DMAs so the final transfer is small
                        nc.gpsimd.dma_start(
                            yt_d[s][:, h - 1 : h + 1, :], ywide[:, h - 1 : h + 1, :]
                        )

    nc.finalize()
    _BUILD_CACHE[key] = nc
    return nc


def _install_trace_shims():
    """Make trace=True usable in this image: provide the NTFF hook module and
    neutralize the artifact upload (no bucket access needed for local use)."""
    import sys
    import types

    try:
        import antenv.axon_hooks  # noqa: F401
    except ImportError:
        hook = None
        try:
            from trn_agent_boot.trn_boot import _ntff_profile_via_ctypes

            hook = _ntff_profile_via_ctypes("/opt/axon/libaxon_pjrt.so")
        except Exception:
            hook = None
        mod = types.ModuleType("antenv.axon_hooks")
        mod._hook = hook
        mod.get_axon_ntff_profile_hook = lambda: mod._hook
        mod.set_axon_ntff_profile_hook = lambda h: setattr(mod, "_hook", h)
        sys.modules["antenv.axon_hooks"] = mod

    import concourse.bass_utils as bu

    orig_upload = bu.upload_artifacts

    def safe_upload(tmpdir):
        try:
            return orig_upload(tmpdir)
        except Exception:
            return tmpdir
    bu.upload_artifacts = safe_upload


def kernel(**inputs):
    import ml_dtypes

    from concourse.bass_utils import run_bass_kernel_spmd

    F8NP = ml_dtypes.float8_e4m3

    hidden = np.ascontiguousarray(np.asarray(inputs["hidden_states"], dtype=np.float32))
    idx = np.asarray(inputs["top_k_index"]).astype(np.int64)
    wts = np.asarray(inputs["top_k_weights"], dtype=np.float32)
    gup = np.asarray(inputs["gate_up_proj"], dtype=np.float32)
    dnp = np.asarray(inputs["down_proj"], dtype=np.float32)

    n_tok = hidden.shape[0]
    K = idx.shape[1]

    ve, vt, vw, vp, va, zero_w = _route(idx, wts, n_tok)
    cnts = np.bincount(ve, minlength=R)
    estarts = np.cumsum(cnts) - cnts

    # tiered slot widths: slot j holds experts ranked [8j, 8j+8) by load
    rank = np.argsort(-cnts, kind="stable")
    widths = []
    for j in range(N_SLOTS):
        w = int(((cnts[rank[8 * j]] + 15) // 16) * 16)
        widths.append(max(64, min(WMAX, w)))
    assert cnts.max() <= WMAX, "expert load exceeds 512; unsupported"
    WTOT = sum(widths)
    xoffs = [sum(widths[:s]) for s in range(N_SLOTS)]
    # slot_expert[c][s] = global expert id
    slot_expert = [[int(rank[8 * s + c]) for s in range(N_SLOTS)] for c in range(N_CORES)]

    xq = hidden.astype(F8NP)  # [T, H] quantized once

    in_maps = []
    for c in range(N_CORES):
        xt = np.zeros((HT, 128, WTOT), dtype=F8NP)
        guw = np.empty((N_SLOTS, 128, HT, 1024), dtype=F8NP)
        dnw = np.empty((N_SLOTS, 128, IT, 1024), dtype=F8NP)
        for s in range(N_SLOTS):
            ge = slot_expert[c][s]
            s0, cnt = estarts[ge], cnts[ge]
            if cnt:
                toks = vt[s0 : s0 + cnt]
                # [cnt, H] -> [H, cnt] -> [HT, 128, cnt]
                xb = xq[toks].T.reshape(HT, 128, cnt)
                xt[:, :, xoffs[s] : xoffs[s] + cnt] = xb
            # guT[h, m] tiles: [128p, HT, 1024m]
            guw[s] = (gup[ge].T.reshape(HT, 128, 1024) * SW).astype(F8NP).transpose(1, 0, 2)
            # dnT[i, h] tiles: [128p, IT, 1024h]
            dnw[s] = (dnp[ge].T.reshape(IT, 128, 1024) * SW).astype(F8NP).transpose(1, 0, 2)
        # pair-major x: [pair, 128, h01, WTOT]
        xt = xt.reshape(4, 2, 128, WTOT).transpose(0, 2, 1, 3)
        in_maps.append({
            "xt": np.ascontiguousarray(xt),
            "guw": np.ascontiguousarray(guw),
            "dnw": np.ascontiguousarray(dnw),
        })

    nc = _build_bass(widths)

    trace = bool(int(os.environ.get("KERNEL_TRACE", "0")))
    if trace:
        _install_trace_shims()
    res = run_bass_kernel_spmd(nc, in_maps, list(range(N_CORES)), trace=trace)
    LAST_RUN["exec_time_ns"] = res.exec_time_ns
    LAST_RUN["mean_exec_time_ns"] = res.mean_exec_time_ns
    LAST_RUN["instructions_and_trace"] = res.instructions_and_trace
    LAST_RUN["profile_json"] = res.profile_json

    # ---- combine on host (descale by SW*SM folded into router weight) ----
    out = hidden * zero_w[:, None].astype(np.float32)
    acc = np.zeros((n_tok * K, H), dtype=np.float32)
    descale = 1.0 / (SW * SM)
    for c in range(N_CORES):
        yt = np.asarray(res.results[c]["yt"]).astype(np.float32)  # [4, 128, HT, 512]
        for s in range(N_SLOTS):
            ge = slot_expert[c][s]
            s0, cnt = estarts[ge], cnts[ge]
            if cnt == 0:
                continue
            y = yt[s].transpose(1, 0, 2).reshape(H, WMAX)[:, :cnt].T  # [cnt, H]
            acc[va[s0 : s0 + cnt]] = y * (vw[s0 : s0 + cnt, None] * descale)
    out += acc.reshape(n_tok, K, H).sum(axis=1)
    return out
